# revision 29
# baseline (speedup 1.0000x reference)
"""Batched per-sample video color augmentation (brightness / contrast /
grayscale / hue / identity / saturation) on 8 Trainium2 NeuronCores.

Sharding: by frame (T=8 == 8 cores). Core j processes images[:, :, j, :, :]
for all 8 samples, so the contrast per-frame mean is core-local and the
expensive hue samples spread evenly across cores.

The Bass program is specialized at build time on the values of
selected_augs / hue_factors / blend_factors (tiny per-sample scalars),
which are folded in as immediates; only the image pixels flow through
the kernel.  Identity samples never touch the device (host copy).

Host pre-permutes each sample-frame to [128 partitions, 3*512] so every
DMA line is 6KB contiguous.  Engine placement (measured at [128,512]
f32): DVE tensor_tensor ~0.69us, DVE tensor_scalar (2x_2P) ~0.27us,
ACT ~0.72-0.84us, GpSimd tensor_scalar ~7.4us (avoid for big tiles).
"""

import sys

import numpy as np

if "/opt/trn_rl_repo" not in sys.path:
    sys.path.insert(0, "/opt/trn_rl_repo")

N_SAMPLES = 8
N_CH = 3
N_FRAMES = 8
H = 256
W = 256
HW = H * W          # 65536
P = 128             # SBUF partitions
F = HW // P         # 512 floats per partition per channel
N_CORES = 8

GRAY_R = 0.2989
GRAY_G = 0.587
GRAY_B = 0.114

_PROGRAM_CACHE: dict = {}


def _build_program(augs, hf, bf):
    """augs: list[int] (identity samples excluded by caller's mapping),
    hf/bf: per-sample float32.  The program reads x[NA,128,1536] and
    writes y[NA,128,1536] where NA = number of non-identity samples."""
    import concourse.tile as tile
    from concourse import bacc, bass_isa, mybir

    dt = mybir.dt
    Alu = mybir.AluOpType
    Act = mybir.ActivationFunctionType
    f32 = np.float32

    active = [n for n in range(len(augs)) if int(augs[n]) != 4]
    na = len(active)
    slot = {n: i for i, n in enumerate(active)}

    nc = bacc.Bacc(
        "TRN2", target_bir_lowering=False, debug=False, num_devices=N_CORES
    )
    x = nc.dram_tensor(
        "x", [na, P, N_CH * F], dt.float32, kind="ExternalInput"
    ).ap()
    y = nc.dram_tensor(
        "y", [na, P, N_CH * F], dt.float32, kind="ExternalOutput"
    ).ap()

    c1 = float(f32(GRAY_R) / f32(GRAY_G))
    c2 = float(f32(GRAY_B) / f32(GRAY_G))

    with tile.TileContext(nc) as tc:
        with (
            tc.tile_pool(name="io", bufs=7) as io_pool,
            tc.tile_pool(name="work", bufs=2) as work,
            tc.tile_pool(name="huep", bufs=1) as huep,
            tc.tile_pool(name="small", bufs=2) as small,
            tc.tile_pool(name="consts", bufs=1) as consts,
        ):
            const_tiles: dict = {}

            def cbias(val):
                v = float(f32(val))
                ct = const_tiles.get(v)
                if ct is None:
                    nm = f"cst{len(const_tiles)}"
                    ct = consts.tile([P, 1], dt.float32, name=nm, tag=nm)
                    nc.gpsimd.memset(ct[:], v)
                    const_tiles[v] = ct
                return ct[:]

            def wtile(nm, shape=(P, F)):
                return work.tile(list(shape), dt.float32, name=nm, tag=nm)

            def htile(nm, shape=(P, F)):
                return huep.tile(list(shape), dt.float32, name=nm, tag=nm)

            def stile(nm):
                return small.tile([P, 1], dt.float32, name=nm, tag=nm)

            def yv(n):      # [P, 3, F] view of sample n's output
                return y[slot[n]].rearrange("p (c f) -> p c f", c=N_CH)

            def ych(n, ci):  # [P, F] view of one output channel
                return yv(n)[:, ci]

            hue_n = [n for n in range(len(augs)) if int(augs[n]) == 3]
            contrast_n = [n for n in range(len(augs)) if int(augs[n]) == 1]
            sat_n = [n for n in range(len(augs)) if int(augs[n]) == 5]
            gray_n = [n for n in range(len(augs)) if int(augs[n]) == 2]
            bright_n = [n for n in range(len(augs)) if int(augs[n]) == 0]

            # --- phase 1: loads (hue first: longest dependent chain).
            # Per-channel DMAs so compute can start as soon as the first
            # channels land (subtile deps track per-slice writes). ---
            xts = {}
            for i_n, n in enumerate(hue_n + contrast_n + sat_n + gray_n + bright_n):
                xt = io_pool.tile([P, N_CH, F], dt.float32, name="xt", tag="xt")
                if i_n == 0 and n in hue_n:
                    # r+g then b: the hue chain's first ops need only r,g
                    nc.sync.dma_start(out=xt[:, 0:2], in_=x[slot[n], :, 0:2 * F])
                    nc.sync.dma_start(out=xt[:, 2], in_=x[slot[n], :, 2 * F:])
                else:
                    nc.sync.dma_start(out=xt[:], in_=x[slot[n]])
                xts[n] = xt

            # --- phase 2: contrast heads (ACT row-sums -> mean scalar) ---
            contrast_m1 = {}
            for n in contrast_n:
                f = float(f32(bf[n]))
                xt = xts[n]
                scr = wtile("actscr")
                rs = []
                for ci in range(N_CH):
                    rsc = stile(f"rs{ci}")
                    nc.scalar.activation(
                        scr[:], xt[:, ci], Act.Identity,
                        bias=0.0, scale=1.0, accum_out=rsc[:],
                    )
                    rs.append(rsc)
                s1 = stile("s1")
                nc.vector.scalar_tensor_tensor(
                    s1[:], rs[0][:], c1, rs[1][:], Alu.mult, Alu.add
                )
                s2 = stile("s2")
                nc.vector.scalar_tensor_tensor(
                    s2[:], rs[2][:], c2, s1[:], Alu.mult, Alu.add
                )
                tot = stile("tot")
                nc.gpsimd.partition_all_reduce(
                    tot[:], s2[:], channels=P,
                    reduce_op=bass_isa.ReduceOp.add,
                )
                # m1 = (1-f)*mean(gray) = tot * 0.587*(1-f)/65536
                m1 = stile("m1")
                m1_imm = float(f32(f32(GRAY_G) * (f32(1.0) - f32(f)) / f32(HW)))
                nc.gpsimd.tensor_scalar(m1[:], tot[:], m1_imm, None, Alu.mult)
                contrast_m1[n] = m1

            # --- phase 3: hue heads.  Division-free form: with D the
            # masked numerator (g-b | b-r | r-g for r|g|b == maxc) and
            # E = D + 6*hf*cr, the three output values are
            #   o_next = minc + min(relu(E), cr)
            #   o_prev = minc + min(relu(-E), cr)
            #   o_max  = minc + (cr - relu(|E| - cr))
            # assigned to channels cyclically from the max channel. ---
            hue_state = {}
            for n in hue_n:
                xt = xts[n]
                r, g, b = xt[:, 0], xt[:, 1], xt[:, 2]
                mx = htile("mx")
                nc.vector.tensor_max(mx[:], r, g)
                v = htile("v")
                nc.vector.tensor_max(v[:], mx[:], b)
                mn = htile("mn")
                nc.vector.tensor_tensor(mn[:], r, g, Alu.min)
                minc = htile("minc")
                nc.vector.tensor_tensor(minc[:], mn[:], b, Alu.min)
                cr = htile("cr")
                nc.vector.tensor_sub(cr[:], v[:], minc[:])
                er = huep.tile([P, F], dt.uint32, name="er", tag="er")
                nc.vector.tensor_tensor(er[:], v[:], r, Alu.is_equal)
                eg = huep.tile([P, F], dt.uint32, name="eg", tag="eg")
                nc.vector.tensor_tensor(eg[:], v[:], g, Alu.is_equal)
                gb = htile("gb")
                nc.vector.tensor_sub(gb[:], g, b)
                br_ = htile("br_")
                nc.vector.tensor_sub(br_[:], b, r)
                dd = htile("dd")
                nc.vector.tensor_sub(dd[:], r, g)        # b-max case: r-g
                nc.vector.copy_predicated(dd[:], eg[:], br_[:])
                nc.vector.copy_predicated(dd[:], er[:], gb[:])
                hf6 = float(f32(6.0) * f32(hf[n]))
                ee = htile("ee")  # E = D + hf6*cr
                nc.vector.scalar_tensor_tensor(
                    ee[:], cr[:], hf6, dd[:], Alu.mult, Alu.add
                )
                p1 = htile("p1")  # relu(E)   (ACT)
                nc.scalar.activation(p1[:], ee[:], Act.Relu, bias=0.0, scale=1.0)
                p2 = htile("p2")  # relu(-E)  (ACT)
                nc.scalar.activation(p2[:], ee[:], Act.Relu, bias=0.0, scale=-1.0)
                hue_state[n] = (p1, p2, cr, minc, v, er, eg)

            # --- phase 4: contrast outs.  f<=1 samples (no clip needed) run
            # on ACT (Identity(x*f + m1)); clip samples use DVE fast TS ---
            for n in contrast_n:
                f = float(f32(bf[n]))
                m1 = contrast_m1[n]
                xt = xts[n]
                yt = io_pool.tile([P, N_CH, F], dt.float32, name="yt", tag="yt")
                nc.vector.tensor_scalar(
                    yt[:], xt[:], f, m1[:], Alu.mult, Alu.add
                )
                if f > 1.0:
                    nc.vector.tensor_scalar(
                        yt[:], yt[:], 0.0, 1.0, Alu.max, Alu.min
                    )
                nc.sync.dma_start(out=yv(n), in_=yt[:])

            # --- phase 4b: saturation ---
            for n in sat_n:
                f = float(f32(bf[n]))
                xt = xts[n]
                r, g, b = xt[:, 0], xt[:, 1], xt[:, 2]
                t1 = wtile("t1")
                nc.vector.scalar_tensor_tensor(
                    t1[:], r, c1, g, Alu.mult, Alu.add
                )
                t2 = wtile("t2")
                nc.vector.scalar_tensor_tensor(
                    t2[:], b, c2, t1[:], Alu.mult, Alu.add
                )
                g1 = wtile("g1")  # (1-f)*gray
                nc.vector.tensor_scalar(
                    g1[:], t2[:],
                    float(f32(GRAY_G) * (f32(1.0) - f32(f))), None, Alu.mult
                )
                yt = io_pool.tile([P, N_CH, F], dt.float32, name="yt", tag="yt")
                for ci, ch in enumerate((r, g, b)):
                    nc.vector.scalar_tensor_tensor(
                        yt[:, ci], ch, f, g1[:], Alu.mult, Alu.add
                    )
                if f > 1.0:
                    nc.vector.tensor_scalar(
                        yt[:], yt[:], 0.0, 1.0, Alu.max, Alu.min
                    )
                nc.sync.dma_start(out=yv(n), in_=yt[:])

            # --- phase 5: hue tails (A-values, mask-mapped to channels,
            # one wide add of minc via broadcast AP, single store) ---
            for n in hue_n:
                p1, p2, cr, minc, v, er, eg = hue_state[n]
                a_next = htile("a_next")
                nc.vector.tensor_tensor(a_next[:], p1[:], cr[:], Alu.min)
                a_prev = htile("a_prev")
                nc.vector.tensor_tensor(a_prev[:], p2[:], cr[:], Alu.min)
                s12 = htile("s12")  # |E|
                nc.vector.tensor_add(s12[:], p1[:], p2[:])
                t1e = htile("t1e")  # |E| - cr
                nc.vector.tensor_sub(t1e[:], s12[:], cr[:])
                t2e = htile("t2e")  # relu(|E| - cr)  (ACT)
                nc.scalar.activation(t2e[:], t1e[:], Act.Relu, bias=0.0, scale=1.0)
                a_max = htile("a_max")  # cr - relu(|E|-cr)
                nc.vector.tensor_sub(a_max[:], cr[:], t2e[:])
                # channel map: default (b max): (R,G,B)=(next,prev,max);
                # g max: (prev,max,next); r max: (max,next,prev)
                a_all = htile("a_all", (P, N_CH, F))
                defaults = (a_next, a_prev, a_max)
                on_eg = (a_prev, a_max, a_next)
                on_er = (a_max, a_next, a_prev)
                for ci in range(N_CH):
                    nc.scalar.copy(a_all[:, ci], defaults[ci][:])
                for ci in range(N_CH):
                    nc.vector.copy_predicated(a_all[:, ci], eg[:], on_eg[ci][:])
                    nc.vector.copy_predicated(a_all[:, ci], er[:], on_er[ci][:])
                    ho = htile(f"ho{ci}")
                    nc.vector.tensor_add(ho[:], a_all[:, ci], minc[:])
                    nc.sync.dma_start(out=ych(n, ci), in_=ho[:])

            # --- phase 5b: gray + brightness ---
            for n in gray_n:
                xt = xts[n]
                r, g, b = xt[:, 0], xt[:, 1], xt[:, 2]
                t1 = wtile("t1")
                nc.vector.scalar_tensor_tensor(
                    t1[:], r, c1, g, Alu.mult, Alu.add
                )
                t2 = wtile("t2")
                nc.vector.scalar_tensor_tensor(
                    t2[:], b, c2, t1[:], Alu.mult, Alu.add
                )
                gray = wtile("gray")
                nc.vector.tensor_scalar(
                    gray[:], t2[:], float(f32(GRAY_G)), None, Alu.mult
                )
                # store one channel only; host replicates to the other two
                nc.sync.dma_start(out=ych(n, 0), in_=gray[:])
            for n in bright_n:
                f = float(f32(bf[n]))
                xt = xts[n]
                yt = io_pool.tile([P, N_CH, F], dt.float32, name="yt", tag="yt")
                nc.vector.tensor_scalar(
                    yt[:], xt[:], f, 1.0, Alu.mult, Alu.min
                )
                nc.sync.dma_start(out=yv(n), in_=yt[:])

    nc.compile()
    return nc


def _get_program(augs, hf, bf):
    key = (tuple(int(v) for v in augs),
           tuple(np.float32(v).tobytes() for v in hf),
           tuple(np.float32(v).tobytes() for v in bf))
    prog = _PROGRAM_CACHE.get(key)
    if prog is None:
        prog = _build_program(augs, hf, bf)
        _PROGRAM_CACHE[key] = prog
    return prog


def _run(images, selected_augs, hue_factors, blend_factors, trace=False):
    from concourse.bass_utils import run_bass_kernel_spmd

    imgs = np.ascontiguousarray(np.asarray(images, dtype=np.float32))
    augs = np.asarray(selected_augs).astype(np.int64)
    hf = np.asarray(hue_factors, dtype=np.float32)
    bf = np.asarray(blend_factors, dtype=np.float32)
    assert imgs.shape == (N_SAMPLES, N_CH, N_FRAMES, H, W), imgs.shape

    active = [n for n in range(N_SAMPLES) if int(augs[n]) != 4]
    out = np.empty((N_SAMPLES, N_CH, N_FRAMES, H, W), dtype=np.float32)
    for n in range(N_SAMPLES):
        if int(augs[n]) == 4:  # identity: out = clip(x) = x, pure copy
            out[n] = imgs[n]

    kres = None
    if active:
        nc = _get_program(augs, hf, bf)
        in_maps = []
        for j in range(N_CORES):
            # [NA, 3, 128, 512] -> [NA, 128, 3, 512]: 6KB-contiguous lines
            xj = imgs[active, :, j].reshape(len(active), N_CH, P, F)
            xj = np.ascontiguousarray(xj.transpose(0, 2, 1, 3)).reshape(
                len(active), P, N_CH * F
            )
            in_maps.append({"x": xj})

        kres = run_bass_kernel_spmd(
            nc, in_maps, list(range(N_CORES)), trace=trace,
            trace_cores=list(range(N_CORES)) if trace else None,
        )

        gray_slots = [i for i, n in enumerate(active) if int(augs[n]) == 2]
        for j in range(N_CORES):
            yj = kres.results[j]["y"].reshape(len(active), P, N_CH, F)
            if gray_slots:
                yj = yj.copy()
            for i in gray_slots:  # device wrote channel 0 only
                yj[i, :, 1] = yj[i, :, 0]
                yj[i, :, 2] = yj[i, :, 0]
            yj = yj.transpose(0, 2, 1, 3).reshape(len(active), N_CH, H, W)
            out[active, :, j] = yj
    return out, kres


def kernel(images, selected_augs, hue_factors, blend_factors):
    out, _ = _run(images, selected_augs, hue_factors, blend_factors, trace=False)
    return out


# revision 30
# speedup vs baseline: 1.0697x; 1.0697x over previous
"""Batched per-sample video color augmentation (brightness / contrast /
grayscale / hue / identity / saturation) on 8 Trainium2 NeuronCores.

Sharding: by frame (T=8 == 8 cores). Core j processes images[:, :, j, :, :]
for all 8 samples, so the contrast per-frame mean is core-local and the
expensive hue samples spread evenly across cores.

The Bass program is specialized at build time on the values of
selected_augs / hue_factors / blend_factors (tiny per-sample scalars),
which are folded in as immediates; only the image pixels flow through
the kernel.  Identity samples never touch the device (host copy).

Host pre-permutes each sample-frame to [128 partitions, 3*512] so every
DMA line is 6KB contiguous.  Engine placement (measured at [128,512]
f32): DVE tensor_tensor ~0.69us, DVE tensor_scalar (2x_2P) ~0.27us,
ACT ~0.72-0.84us, GpSimd tensor_scalar ~7.4us (avoid for big tiles).
"""

import sys

import numpy as np

if "/opt/trn_rl_repo" not in sys.path:
    sys.path.insert(0, "/opt/trn_rl_repo")

N_SAMPLES = 8
N_CH = 3
N_FRAMES = 8
H = 256
W = 256
HW = H * W          # 65536
P = 128             # SBUF partitions
F = HW // P         # 512 floats per partition per channel
N_CORES = 8

GRAY_R = 0.2989
GRAY_G = 0.587
GRAY_B = 0.114

_PROGRAM_CACHE: dict = {}


def _build_program(augs, hf, bf):
    """augs: list[int] (identity samples excluded by caller's mapping),
    hf/bf: per-sample float32.  The program reads x[NA,128,1536] and
    writes y[NA,128,1536] where NA = number of non-identity samples."""
    import concourse.tile as tile
    from concourse import bacc, bass_isa, mybir

    dt = mybir.dt
    Alu = mybir.AluOpType
    Act = mybir.ActivationFunctionType
    f32 = np.float32

    active = [n for n in range(len(augs)) if int(augs[n]) != 4]
    na = len(active)
    slot = {n: i for i, n in enumerate(active)}

    nc = bacc.Bacc(
        "TRN2", target_bir_lowering=False, debug=False, num_devices=N_CORES
    )
    x = nc.dram_tensor(
        "x", [na, P, N_CH * F], dt.float32, kind="ExternalInput"
    ).ap()
    y = nc.dram_tensor(
        "y", [na, P, N_CH * F], dt.float32, kind="ExternalOutput"
    ).ap()

    c1 = float(f32(GRAY_R) / f32(GRAY_G))
    c2 = float(f32(GRAY_B) / f32(GRAY_G))

    with tile.TileContext(nc) as tc:
        with (
            tc.tile_pool(name="io", bufs=7) as io_pool,
            tc.tile_pool(name="work", bufs=2) as work,
            tc.tile_pool(name="huep", bufs=1) as huep,
            tc.tile_pool(name="small", bufs=2) as small,
            tc.tile_pool(name="consts", bufs=1) as consts,
        ):
            const_tiles: dict = {}

            def cbias(val):
                v = float(f32(val))
                ct = const_tiles.get(v)
                if ct is None:
                    nm = f"cst{len(const_tiles)}"
                    ct = consts.tile([P, 1], dt.float32, name=nm, tag=nm)
                    nc.gpsimd.memset(ct[:], v)
                    const_tiles[v] = ct
                return ct[:]

            def wtile(nm, shape=(P, F)):
                return work.tile(list(shape), dt.float32, name=nm, tag=nm)

            def htile(nm, shape=(P, F)):
                return huep.tile(list(shape), dt.float32, name=nm, tag=nm)

            def stile(nm):
                return small.tile([P, 1], dt.float32, name=nm, tag=nm)

            def yv(n):      # [P, 3, F] view of sample n's output
                return y[slot[n]].rearrange("p (c f) -> p c f", c=N_CH)

            def ych(n, ci):  # [P, F] view of one output channel
                return yv(n)[:, ci]

            hue_n = [n for n in range(len(augs)) if int(augs[n]) == 3]
            contrast_n = [n for n in range(len(augs)) if int(augs[n]) == 1]
            sat_n = [n for n in range(len(augs)) if int(augs[n]) == 5]
            gray_n = [n for n in range(len(augs)) if int(augs[n]) == 2]
            bright_n = [n for n in range(len(augs)) if int(augs[n]) == 0]

            # --- phase 1: loads (hue first: longest dependent chain).
            # Per-channel DMAs so compute can start as soon as the first
            # channels land (subtile deps track per-slice writes). ---
            xts = {}
            for i_n, n in enumerate(hue_n + contrast_n + sat_n + gray_n + bright_n):
                xt = io_pool.tile([P, N_CH, F], dt.float32, name="xt", tag="xt")
                if i_n == 0 and n in hue_n:
                    # r+g then b: the hue chain's first ops need only r,g
                    nc.sync.dma_start(out=xt[:, 0:2], in_=x[slot[n], :, 0:2 * F])
                    nc.sync.dma_start(out=xt[:, 2], in_=x[slot[n], :, 2 * F:])
                else:
                    nc.sync.dma_start(out=xt[:], in_=x[slot[n]])
                xts[n] = xt

            # --- phase 2: contrast heads (ACT row-sums -> mean scalar) ---
            contrast_m1 = {}
            for n in contrast_n:
                f = float(f32(bf[n]))
                xt = xts[n]
                scr = wtile("actscr")
                rs = []
                for ci in range(N_CH):
                    rsc = stile(f"rs{ci}")
                    nc.scalar.activation(
                        scr[:], xt[:, ci], Act.Identity,
                        bias=0.0, scale=1.0, accum_out=rsc[:],
                    )
                    rs.append(rsc)
                s1 = stile("s1")
                nc.vector.scalar_tensor_tensor(
                    s1[:], rs[0][:], c1, rs[1][:], Alu.mult, Alu.add
                )
                s2 = stile("s2")
                nc.vector.scalar_tensor_tensor(
                    s2[:], rs[2][:], c2, s1[:], Alu.mult, Alu.add
                )
                tot = stile("tot")
                nc.gpsimd.partition_all_reduce(
                    tot[:], s2[:], channels=P,
                    reduce_op=bass_isa.ReduceOp.add,
                )
                # m1 = (1-f)*mean(gray) = tot * 0.587*(1-f)/65536
                m1 = stile("m1")
                m1_imm = float(f32(f32(GRAY_G) * (f32(1.0) - f32(f)) / f32(HW)))
                nc.gpsimd.tensor_scalar(m1[:], tot[:], m1_imm, None, Alu.mult)
                contrast_m1[n] = m1

            # --- phase 3: hue heads.  Division-free form: with D the
            # masked numerator (g-b | b-r | r-g for r|g|b == maxc) and
            # E = D + 6*hf*cr, the three output values are
            #   o_next = minc + min(relu(E), cr)
            #   o_prev = minc + min(relu(-E), cr)
            #   o_max  = minc + (cr - relu(|E| - cr))
            # assigned to channels cyclically from the max channel. ---
            hue_state = {}
            for n in hue_n:
                xt = xts[n]
                r, g, b = xt[:, 0], xt[:, 1], xt[:, 2]
                mx = htile("mx")
                nc.vector.tensor_max(mx[:], r, g)
                v = htile("v")
                nc.vector.tensor_max(v[:], mx[:], b)
                mn = htile("mn")
                nc.vector.tensor_tensor(mn[:], r, g, Alu.min)
                minc = htile("minc")
                nc.vector.tensor_tensor(minc[:], mn[:], b, Alu.min)
                cr = htile("cr")
                nc.vector.tensor_sub(cr[:], v[:], minc[:])
                er = huep.tile([P, F], dt.uint32, name="er", tag="er")
                nc.vector.tensor_tensor(er[:], v[:], r, Alu.is_equal)
                eg = huep.tile([P, F], dt.uint32, name="eg", tag="eg")
                nc.vector.tensor_tensor(eg[:], v[:], g, Alu.is_equal)
                gb = htile("gb")
                nc.vector.tensor_sub(gb[:], g, b)
                br_ = htile("br_")
                nc.vector.tensor_sub(br_[:], b, r)
                dd = htile("dd")
                nc.vector.tensor_sub(dd[:], r, g)        # b-max case: r-g
                nc.vector.copy_predicated(dd[:], eg[:], br_[:])
                nc.vector.copy_predicated(dd[:], er[:], gb[:])
                hf6 = float(f32(6.0) * f32(hf[n]))
                ee = htile("ee")  # E = D + hf6*cr
                nc.vector.scalar_tensor_tensor(
                    ee[:], cr[:], hf6, dd[:], Alu.mult, Alu.add
                )
                p1 = htile("p1")  # relu(E)   (ACT)
                nc.scalar.activation(p1[:], ee[:], Act.Relu, bias=0.0, scale=1.0)
                p2 = htile("p2")  # relu(-E)  (ACT)
                nc.scalar.activation(p2[:], ee[:], Act.Relu, bias=0.0, scale=-1.0)
                hue_state[n] = (p1, p2, cr, minc, v, er, eg)

            # --- phase 4: contrast outs.  f<=1 samples (no clip needed) run
            # on ACT (Identity(x*f + m1)); clip samples use DVE fast TS ---
            for n in contrast_n:
                f = float(f32(bf[n]))
                m1 = contrast_m1[n]
                xt = xts[n]
                yt = io_pool.tile([P, N_CH, F], dt.float32, name="yt", tag="yt")
                if f > 1.0:
                    nc.vector.tensor_scalar(
                        yt[:], xt[:], f, m1[:], Alu.mult, Alu.add
                    )
                    nc.vector.tensor_scalar(
                        yt[:], yt[:], 0.0, 1.0, Alu.max, Alu.min
                    )
                else:
                    # no clip needed: convex combination stays in [0,1);
                    # run on ACT to offload DVE
                    nc.scalar.activation(
                        yt[:], xt[:], Act.Identity, bias=m1[:], scale=f
                    )
                nc.sync.dma_start(out=yv(n), in_=yt[:])

            # --- phase 4b: saturation ---
            for n in sat_n:
                f = float(f32(bf[n]))
                xt = xts[n]
                r, g, b = xt[:, 0], xt[:, 1], xt[:, 2]
                t1 = wtile("t1")
                nc.vector.scalar_tensor_tensor(
                    t1[:], r, c1, g, Alu.mult, Alu.add
                )
                t2 = wtile("t2")
                nc.vector.scalar_tensor_tensor(
                    t2[:], b, c2, t1[:], Alu.mult, Alu.add
                )
                g1 = wtile("g1")  # (1-f)*gray
                nc.vector.tensor_scalar(
                    g1[:], t2[:],
                    float(f32(GRAY_G) * (f32(1.0) - f32(f))), None, Alu.mult
                )
                yt = io_pool.tile([P, N_CH, F], dt.float32, name="yt", tag="yt")
                for ci, ch in enumerate((r, g, b)):
                    nc.vector.scalar_tensor_tensor(
                        yt[:, ci], ch, f, g1[:], Alu.mult, Alu.add
                    )
                if f > 1.0:
                    nc.vector.tensor_scalar(
                        yt[:], yt[:], 0.0, 1.0, Alu.max, Alu.min
                    )
                nc.sync.dma_start(out=yv(n), in_=yt[:])

            # --- phase 5: hue tails (A-values, mask-mapped to channels,
            # one wide add of minc via broadcast AP, single store) ---
            for n in hue_n:
                p1, p2, cr, minc, v, er, eg = hue_state[n]
                a_next = htile("a_next")
                nc.vector.tensor_tensor(a_next[:], p1[:], cr[:], Alu.min)
                a_prev = htile("a_prev")
                nc.vector.tensor_tensor(a_prev[:], p2[:], cr[:], Alu.min)
                s12 = htile("s12")  # |E|
                nc.vector.tensor_add(s12[:], p1[:], p2[:])
                t1e = htile("t1e")  # |E| - cr
                nc.vector.tensor_sub(t1e[:], s12[:], cr[:])
                t2e = htile("t2e")  # relu(|E| - cr)  (ACT)
                nc.scalar.activation(t2e[:], t1e[:], Act.Relu, bias=0.0, scale=1.0)
                a_max = htile("a_max")  # cr - relu(|E|-cr)
                nc.vector.tensor_sub(a_max[:], cr[:], t2e[:])
                # channel map: default (b max): (R,G,B)=(next,prev,max);
                # g max: (prev,max,next); r max: (max,next,prev)
                a_all = htile("a_all", (P, N_CH, F))
                defaults = (a_next, a_prev, a_max)
                on_eg = (a_prev, a_max, a_next)
                on_er = (a_max, a_next, a_prev)
                for ci in range(N_CH):
                    nc.scalar.copy(a_all[:, ci], defaults[ci][:])
                for ci in range(N_CH):
                    nc.vector.copy_predicated(a_all[:, ci], eg[:], on_eg[ci][:])
                    nc.vector.copy_predicated(a_all[:, ci], er[:], on_er[ci][:])
                    ho = htile(f"ho{ci}")
                    nc.vector.tensor_add(ho[:], a_all[:, ci], minc[:])
                    nc.sync.dma_start(out=ych(n, ci), in_=ho[:])

            # --- phase 5b: gray + brightness ---
            for n in gray_n:
                xt = xts[n]
                r, g, b = xt[:, 0], xt[:, 1], xt[:, 2]
                t1 = wtile("t1")
                nc.vector.scalar_tensor_tensor(
                    t1[:], r, c1, g, Alu.mult, Alu.add
                )
                t2 = wtile("t2")
                nc.vector.scalar_tensor_tensor(
                    t2[:], b, c2, t1[:], Alu.mult, Alu.add
                )
                gray = wtile("gray")
                nc.vector.tensor_scalar(
                    gray[:], t2[:], float(f32(GRAY_G)), None, Alu.mult
                )
                # store one channel only; host replicates to the other two
                nc.sync.dma_start(out=ych(n, 0), in_=gray[:])
            for n in bright_n:
                f = float(f32(bf[n]))
                xt = xts[n]
                yt = io_pool.tile([P, N_CH, F], dt.float32, name="yt", tag="yt")
                nc.vector.tensor_scalar(
                    yt[:], xt[:], f, 1.0, Alu.mult, Alu.min
                )
                nc.sync.dma_start(out=yv(n), in_=yt[:])

    nc.compile()
    return nc


def _get_program(augs, hf, bf):
    key = (tuple(int(v) for v in augs),
           tuple(np.float32(v).tobytes() for v in hf),
           tuple(np.float32(v).tobytes() for v in bf))
    prog = _PROGRAM_CACHE.get(key)
    if prog is None:
        prog = _build_program(augs, hf, bf)
        _PROGRAM_CACHE[key] = prog
    return prog


def _run(images, selected_augs, hue_factors, blend_factors, trace=False):
    from concourse.bass_utils import run_bass_kernel_spmd

    imgs = np.ascontiguousarray(np.asarray(images, dtype=np.float32))
    augs = np.asarray(selected_augs).astype(np.int64)
    hf = np.asarray(hue_factors, dtype=np.float32)
    bf = np.asarray(blend_factors, dtype=np.float32)
    assert imgs.shape == (N_SAMPLES, N_CH, N_FRAMES, H, W), imgs.shape

    active = [n for n in range(N_SAMPLES) if int(augs[n]) != 4]
    out = np.empty((N_SAMPLES, N_CH, N_FRAMES, H, W), dtype=np.float32)
    for n in range(N_SAMPLES):
        if int(augs[n]) == 4:  # identity: out = clip(x) = x, pure copy
            out[n] = imgs[n]

    kres = None
    if active:
        nc = _get_program(augs, hf, bf)
        in_maps = []
        for j in range(N_CORES):
            # [NA, 3, 128, 512] -> [NA, 128, 3, 512]: 6KB-contiguous lines
            xj = imgs[active, :, j].reshape(len(active), N_CH, P, F)
            xj = np.ascontiguousarray(xj.transpose(0, 2, 1, 3)).reshape(
                len(active), P, N_CH * F
            )
            in_maps.append({"x": xj})

        kres = run_bass_kernel_spmd(
            nc, in_maps, list(range(N_CORES)), trace=trace,
            trace_cores=list(range(N_CORES)) if trace else None,
        )

        gray_slots = [i for i, n in enumerate(active) if int(augs[n]) == 2]
        for j in range(N_CORES):
            yj = kres.results[j]["y"].reshape(len(active), P, N_CH, F)
            if gray_slots:
                yj = yj.copy()
            for i in gray_slots:  # device wrote channel 0 only
                yj[i, :, 1] = yj[i, :, 0]
                yj[i, :, 2] = yj[i, :, 0]
            yj = yj.transpose(0, 2, 1, 3).reshape(len(active), N_CH, H, W)
            out[active, :, j] = yj
    return out, kres


def kernel(images, selected_augs, hue_factors, blend_factors):
    out, _ = _run(images, selected_augs, hue_factors, blend_factors, trace=False)
    return out
